# revision 1
# baseline (speedup 1.0000x reference)
"""Trainium2 Bass kernel for nn_ConversationGNN (2-layer GAT, 50K nodes / 500K edges).

Strategy (8 NeuronCores, SPMD, one program):
  - Host: relabel nodes so each core owns 49 windows x 128 nodes, with edges
    (incl. self-loops) bin-packed so every window holds <= F*128 edges. All
    per-core structure (gather indices, one-hot scatter tiles) becomes plain
    input data -> a single uniform program runs on all 8 cores.
  - Device, per core:
      phase A: enc + gat1 linear for own nodes (augmented weights fold the
               attention dot-products a_src/a_dst into extra output columns)
      AllGather the [6272 x 1088] feature+alpha table -> full 50176-row table
      phase C: per edge-tile (128 edges): indirect-DMA gather source rows,
               broadcast dst alpha via one-hot matmul, LeakyReLU+exp on chip,
               softmax-weighted scatter back via one-hot matmul into PSUM
               (numerator and denominator accumulated in one pass), then
               normalize + bias + ELU per 128-dst window; PE-transpose h1 for
               the next layer's matmul.
      phase D: gat2 linear; AllGather table2; phase F: same aggregation for
               layer 2; write output shard.
  - Softmax max-subtraction is skipped: e = leaky(as+ad) is O(+-10) here, so
    exp() is well within f32 range and softmax is shift-invariant.

Self-contained: hardcodes all shapes; only needs numpy + the concourse tree
at /opt/trn_rl_repo (container-provided).
"""

import heapq
import sys

import ml_dtypes

import numpy as np

for _p in ("/opt/trn_rl_repo",):
    if _p not in sys.path:
        sys.path.insert(0, _p)

# problem constants
N = 50000
IN_DIM = 384
HID = 256
HEADS = 4
OUT_DIM = 128
NEG_SLOPE = 0.2

NCORES = 8
P = 128
NW = 49               # dst windows per core
NSH = NW * P          # 6272 padded nodes per core
NT = NCORES * NSH     # 50176 padded nodes total
D1 = HEADS * HID      # 1024
DT1 = 1032            # table1 bf16 row: [h1pre(1024) | alpha_src 4xf32 in 8 slots]
DT2 = 192             # table2 row: [h2pre(128) | alpha_src(1) | pad(63)]
W1C = D1 + 8          # 1032: [W1 | a_src_fold(4) | a_dst_fold(4)]
W2C = OUT_DIM + 2     # 130

# AllGather chunking: windows grouped into chunks; table rows are chunk-major
# (chunk, core, window-in-chunk, slot) so each chunk's AllGather writes one
# contiguous row range and can start as soon as phase A finishes that chunk.
CH_W = [13, 12, 12, 12]
CH_W0 = [0, 13, 25, 37]                      # first window of each chunk
CH_BASE = [0, 13 * 128 * NCORES, 25 * 128 * NCORES, 37 * 128 * NCORES]
CHUNK_OF_W = [0] * 13 + [1] * 12 + [2] * 12 + [3] * 12


def _table_id(padded):
    """Map dst-padded id (core*NSH + w*128 + s) -> chunk-major table row."""
    c = padded // NSH
    r = padded % NSH
    w = r // P
    s = r % P
    k = np.asarray(CHUNK_OF_W)[w]
    base = np.asarray(CH_BASE)[k]
    w0 = np.asarray(CH_W0)[k]
    szk = np.asarray(CH_W)[k]
    return base + c * szk * P + (w - w0) * P + s


# ---------------------------------------------------------------- host side

def _pack_nodes(deg):
    """Assign nodes to (window, slot) so window edge-counts are balanced.

    Returns padded ids [N] (window*128 + slot) and per-window edge counts.
    """
    nwg = NCORES * NW
    order = np.argsort(-deg, kind="stable")
    heap = [(0, w) for w in range(nwg)]
    heapq.heapify(heap)
    slots_used = np.zeros(nwg, np.int64)
    edges_w = np.zeros(nwg, np.int64)
    assign_w = np.empty(N, np.int64)
    assign_s = np.empty(N, np.int64)
    for i in order:
        while True:
            _, w = heapq.heappop(heap)
            if slots_used[w] < P:
                break
        assign_w[i] = w
        assign_s[i] = slots_used[w]
        slots_used[w] += 1
        edges_w[w] += deg[i]
        heapq.heappush(heap, (int(edges_w[w]), w))
    return assign_w * P + assign_s, edges_w


def preprocess(x, node_attr, edge_index, enc_W, enc_b,
               W1, a_src1, a_dst1, b1, W2, a_src2, a_dst2, b2):
    x = np.asarray(x, np.float32)
    node_attr = np.asarray(node_attr, np.float32)
    ei = np.asarray(edge_index)
    src_all = np.concatenate([ei[0], np.arange(N, dtype=ei.dtype)]).astype(np.int64)
    dst_all = np.concatenate([ei[1], np.arange(N, dtype=ei.dtype)]).astype(np.int64)
    ne = src_all.shape[0]

    deg = np.bincount(dst_all, minlength=N) + 0  # self-loops already included
    padded, edges_w = _pack_nodes(deg)
    F = int(np.ceil(edges_w.max() / P))
    T = NW * F

    spad = padded[src_all]
    dpad = padded[dst_all]
    wg = dpad // P
    dst_rel = (dpad % P).astype(np.int64)

    order_e = np.argsort(wg, kind="stable")
    wg_s = wg[order_e]
    counts = np.bincount(wg_s, minlength=NCORES * NW)
    starts = np.zeros(NCORES * NW + 1, np.int64)
    starts[1:] = np.cumsum(counts)
    slot = np.arange(ne) - starts[wg_s]

    core_of = wg_s // NW
    w_loc = wg_s % NW
    tile_g = w_loc * F + slot // P
    e_in = slot % P

    SRC = np.zeros((NCORES, T, P), np.int32)
    SED = np.zeros((NCORES, T, P, P), np.float32)
    SRC[core_of, tile_g, e_in] = spad[order_e].astype(np.int32)
    SED[core_of, tile_g, e_in, dst_rel[order_e]] = 1.0

    x2 = np.zeros((NT, 512), np.float32)
    x2[padded, 0:IN_DIM] = x
    x2[padded, IN_DIM:IN_DIM + 2] = node_attr
    x2[padded, IN_DIM + 2] = 1.0

    encWaug = np.zeros((512, IN_DIM), np.float32)
    encWaug[0:IN_DIM + 2] = np.asarray(enc_W, np.float32)
    encWaug[IN_DIM + 2] = np.asarray(enc_b, np.float32)

    W1 = np.asarray(W1, np.float32)
    asrc1t = np.einsum("fhc,hc->fh", W1.reshape(IN_DIM, HEADS, HID),
                       np.asarray(a_src1, np.float32))
    adst1t = np.einsum("fhc,hc->fh", W1.reshape(IN_DIM, HEADS, HID),
                       np.asarray(a_dst1, np.float32))
    W1aug = np.concatenate([W1, asrc1t, adst1t], axis=1)  # [384, 1032]

    W2 = np.asarray(W2, np.float32)
    W2aug = np.concatenate(
        [W2,
         W2 @ np.asarray(a_src2, np.float32)[0][:, None],
         W2 @ np.asarray(a_dst2, np.float32)[0][:, None]], axis=1)  # [1024, 130]

    b1rep = np.tile(np.asarray(b1, np.float32)[None, :], (P, 1))
    b2rep = np.tile(np.asarray(b2, np.float32)[None, :], (P, 1))

    in_maps = []
    for c in range(NCORES):
        in_maps.append({
            "x2T": np.ascontiguousarray(x2[c * NSH:(c + 1) * NSH].T),
            "encW": encWaug,
            "w1aug": W1aug,
            "w2aug": W2aug,
            "b1rep": b1rep,
            "b2rep": b2rep,
            "srcidx": np.ascontiguousarray(SRC[c].T),       # [128, T]
            "s_both": np.ascontiguousarray(
                np.stack([SED[c], SED[c].transpose(0, 2, 1)], axis=1)
            ).astype(ml_dtypes.bfloat16),
        })
    tid_of = np.arange(NT)  # padded id == table row
    return {"in_maps": in_maps, "F": F, "padded": padded, "tid_of": tid_of}


# -------------------------------------------------------------- bass program

def build_program(F):
    import concourse.bacc as bacc
    import concourse.bass as bass
    import concourse.mybir as mybir
    import concourse.tile as tile
    from concourse.masks import make_identity

    fp32 = mybir.dt.float32
    i32 = mybir.dt.int32
    Alu = mybir.AluOpType
    Act = mybir.ActivationFunctionType
    T = NW * F

    nc = bacc.Bacc("TRN2", target_bir_lowering=False, debug=False,
                   enable_asserts=False, num_devices=NCORES)

    x2T = nc.dram_tensor("x2T", [512, NSH], fp32, kind="ExternalInput")
    encW = nc.dram_tensor("encW", [512, IN_DIM], fp32, kind="ExternalInput")
    w1aug = nc.dram_tensor("w1aug", [IN_DIM, W1C], fp32, kind="ExternalInput")
    w2aug = nc.dram_tensor("w2aug", [D1, W2C], fp32, kind="ExternalInput")
    b1rep = nc.dram_tensor("b1rep", [P, D1], fp32, kind="ExternalInput")
    b2rep = nc.dram_tensor("b2rep", [P, OUT_DIM], fp32, kind="ExternalInput")
    srcidx = nc.dram_tensor("srcidx", [P, T], i32, kind="ExternalInput")
    bf16 = mybir.dt.bfloat16
    s_both = nc.dram_tensor("s_both", [T, 2, P, P], bf16, kind="ExternalInput")
    out = nc.dram_tensor("out", [NSH, OUT_DIM], fp32, kind="ExternalOutput")

    with tile.TileContext(nc) as tc:
        with (
            tc.tile_pool(name="const", bufs=1) as constp,
            tc.tile_pool(name="dram", bufs=1, space="DRAM") as dram,
        ):
            # ---- persistent SBUF data
            enc_sb = constp.tile([P, 4, IN_DIM], fp32)
            nc.sync.dma_start(enc_sb[:], encW.ap().rearrange("(k p) f -> p k f", p=P))
            w1_sb = constp.tile([P, 3, W1C], fp32)
            nc.sync.dma_start(w1_sb[:], w1aug.ap().rearrange("(k p) f -> p k f", p=P))
            w2_sb = constp.tile([P, 8, W2C], fp32)
            nc.sync.dma_start(w2_sb[:], w2aug.ap().rearrange("(k p) f -> p k f", p=P))
            b1_sb = constp.tile([P, D1], fp32)
            nc.sync.dma_start(b1_sb[:], b1rep.ap())
            b2_sb = constp.tile([P, OUT_DIM], fp32)
            nc.sync.dma_start(b2_sb[:], b2rep.ap())
            sidx_sb = constp.tile([P, T], i32)
            nc.sync.dma_start(sidx_sb[:], srcidx.ap())
            ident = constp.tile([P, P], fp32)
            make_identity(nc, ident[:])
            ad1_sb = constp.tile([P, NW, 4], bf16)
            ad2_sb = constp.tile([P, NW], bf16)

            shard1 = dram.tile([NSH, DT1], bf16)
            table1 = dram.tile([NT, DT1], bf16, addr_space="Shared")
            h1T = dram.tile([D1, NSH], fp32)
            shard2 = dram.tile([NSH, DT2], fp32)
            table2 = dram.tile([NT, DT2], fp32, addr_space="Shared")

            # ================= phase A: enc + gat1 linear =================
            with (
                tc.tile_pool(name="pa_sb", bufs=3) as pa,
                tc.tile_pool(name="pa_ps", bufs=1, space="PSUM") as pap,
                tc.tile_pool(name="pa_ps2", bufs=1, space="PSUM") as pap2,
            ):
                for i in range(NW):
                    xt = pa.tile([P, 4, P], fp32, tag="xt")
                    nc.sync.dma_start(
                        xt[:],
                        x2T.ap().rearrange("(k p) n -> p k n", p=P)[
                            :, :, i * P:(i + 1) * P],
                    )
                    ph0 = pap.tile([P, 3, P], fp32, tag="ph0")
                    for j in range(3):
                        for ks in range(4):
                            nc.tensor.matmul(
                                ph0[:, j, :],
                                lhsT=enc_sb[:, ks, j * P:(j + 1) * P],
                                rhs=xt[:, ks, :],
                                start=(ks == 0), stop=(ks == 3),
                            )
                    h0t = pa.tile([P, 3, P], fp32, tag="h0t")
                    nc.vector.tensor_copy(h0t[:], ph0[:])

                    ph1a = pap2.tile([P, 512], fp32, tag="ph1a")
                    ph1b = pap2.tile([P, 512], fp32, tag="ph1b")
                    ph1c = pap2.tile([P, 8], fp32, tag="ph1c")
                    for ks in range(3):
                        st, sp = (ks == 0), (ks == 2)
                        nc.tensor.matmul(ph1a[:], lhsT=h0t[:, ks, :],
                                         rhs=w1_sb[:, ks, 0:512], start=st, stop=sp)
                        nc.tensor.matmul(ph1b[:], lhsT=h0t[:, ks, :],
                                         rhs=w1_sb[:, ks, 512:1024], start=st, stop=sp)
                        nc.tensor.matmul(ph1c[:], lhsT=h0t[:, ks, :],
                                         rhs=w1_sb[:, ks, 1024:1032], start=st, stop=sp)
                    sh1 = pa.tile([P, DT1], bf16, tag="sh1")
                    nc.vector.tensor_copy(sh1[:, 0:512], ph1a[:])
                    nc.vector.tensor_copy(sh1[:, 512:1024], ph1b[:])
                    nc.vector.tensor_copy(
                        sh1[:, 1024:1032].bitcast(mybir.dt.float32), ph1c[:, 0:4])
                    nc.vector.tensor_copy(ad1_sb[:, i, :], ph1c[:, 4:8])
                    nc.sync.dma_start(shard1[i * P:(i + 1) * P, :], sh1[:])

            nc.gpsimd.collective_compute(
                "AllGather", Alu.bypass,
                replica_groups=[list(range(NCORES))],
                ins=[shard1.opt()], outs=[table1.opt()],
            )

            # ================= phase C: gat1 aggregation ==================
            with (
                tc.tile_pool(name="pc_g", bufs=3) as pg,
                tc.tile_pool(name="pc_m", bufs=3) as pm,
                tc.tile_pool(name="pc_s", bufs=4) as psd,
                tc.tile_pool(name="pc_q", bufs=4) as pq,
                tc.tile_pool(name="pc_w", bufs=2) as pw,
                tc.tile_pool(name="pc_po", bufs=2, space="PSUM") as ppo,
                tc.tile_pool(name="pc_sc", bufs=2, space="PSUM") as psc,
            ):
                for w in range(NW):
                    po0 = ppo.tile([P, 512], fp32, tag="po0")
                    po1 = ppo.tile([P, 512], fp32, tag="po1")
                    po2 = ppo.tile([P, 8], fp32, tag="po2")
                    for t in range(F):
                        tg = w * F + t
                        g = pg.tile([P, DT1], bf16, tag="g")
                        nc.gpsimd.indirect_dma_start(
                            out=g[:], out_offset=None, in_=table1[:],
                            in_offset=bass.IndirectOffsetOnAxis(
                                ap=sidx_sb[:, tg:tg + 1], axis=0),
                        )
                        sb2 = psd.tile([P, 2, P], bf16, tag="sboth")
                        nc.sync.dma_start(
                            sb2[:], s_both.ap()[tg].rearrange("j p c -> p j c"))
                        sed = sb2[:, 0, :]
                        sde = sb2[:, 1, :]

                        pead = psc.tile([P, P], fp32, tag="sc")
                        nc.tensor.matmul(pead[:, 0:4], lhsT=sde[:],
                                         rhs=ad1_sb[:, w, :], start=True, stop=True)
                        q = pq.tile([P, 4], fp32, tag="q")
                        q2 = pq.tile([P, 4], fp32, tag="q2")
                        nc.vector.tensor_add(
                            q[:], g[:, 1024:1032].bitcast(mybir.dt.float32),
                            pead[:, 0:4])
                        nc.vector.tensor_scalar_mul(q2[:], q[:], NEG_SLOPE)
                        nc.vector.tensor_tensor(q[:], q[:], q2[:], op=Alu.max)
                        msg = pm.tile([P, D1 + 4], bf16, tag="msg")
                        nc.scalar.activation(msg[:, D1:D1 + 4], q[:], Act.Exp)
                        nc.vector.tensor_tensor(
                            out=msg[:, 0:D1].rearrange("p (h c) -> p h c", h=HEADS),
                            in0=g[:, 0:D1].rearrange("p (h c) -> p h c", h=HEADS),
                            in1=msg[:, D1:D1 + 4][:, :, None].to_broadcast(
                                [P, HEADS, HID]),
                            op=Alu.mult,
                        )
                        st, sp = (t == 0), (t == F - 1)
                        nc.tensor.matmul(po0[:], lhsT=sed[:], rhs=msg[:, 0:512],
                                         start=st, stop=sp)
                        nc.tensor.matmul(po1[:], lhsT=sed[:], rhs=msg[:, 512:1024],
                                         start=st, stop=sp)
                        nc.tensor.matmul(po2[:, 0:4], lhsT=sed[:],
                                         rhs=msg[:, 1024:1028], start=st, stop=sp)
                    # ---- window drain: softmax-normalize, bias, ELU
                    rden = pq.tile([P, 4], fp32, tag="rden")
                    nc.vector.tensor_scalar_add(rden[:], po2[:, 0:4], 1e-16)
                    nc.vector.reciprocal(rden[:], rden[:])
                    h1 = pw.tile([P, D1], fp32, tag="h1")
                    nc.vector.tensor_tensor(
                        out=h1[:, 0:512].rearrange("p (h c) -> p h c", h=2),
                        in0=po0[:].rearrange("p (h c) -> p h c", h=2),
                        in1=rden[:, 0:2][:, :, None].to_broadcast([P, 2, HID]),
                        op=Alu.mult)
                    nc.vector.tensor_tensor(
                        out=h1[:, 512:1024].rearrange("p (h c) -> p h c", h=2),
                        in0=po1[:].rearrange("p (h c) -> p h c", h=2),
                        in1=rden[:, 2:4][:, :, None].to_broadcast([P, 2, HID]),
                        op=Alu.mult)
                    nc.vector.tensor_add(h1[:], h1[:], b1_sb[:])
                    # ELU(x) = max(x,0) + exp(min(x,0)) - 1
                    em = pw.tile([P, D1], fp32, tag="em")
                    nc.vector.tensor_scalar_min(em[:], h1[:], 0.0)
                    nc.scalar.activation(em[:], em[:], Act.Exp)
                    nc.vector.tensor_scalar_max(h1[:], h1[:], 0.0)
                    nc.vector.tensor_add(h1[:], h1[:], em[:])
                    nc.vector.tensor_scalar_add(h1[:], h1[:], -1.0)
                    # transpose to feature-major for phase D
                    trs = pw.tile([P, 8, P], fp32, tag="trs")
                    for fb in range(8):
                        ptr = psc.tile([P, P], fp32, tag="sc")
                        nc.tensor.transpose(ptr[:], h1[:, fb * P:(fb + 1) * P],
                                            ident[:])
                        nc.vector.tensor_copy(trs[:, fb, :], ptr[:])
                    nc.sync.dma_start(
                        h1T[:].rearrange("(k p) n -> p k n", p=P)[
                            :, :, w * P:(w + 1) * P],
                        trs[:])

            # ================= phase D: gat2 linear =======================
            with (
                tc.tile_pool(name="pd_sb", bufs=3) as pd,
                tc.tile_pool(name="pd_ps", bufs=2, space="PSUM") as pdp,
            ):
                for i in range(NW):
                    ht = pd.tile([P, 8, P], fp32, tag="ht")
                    nc.sync.dma_start(
                        ht[:],
                        h1T[:].rearrange("(k p) n -> p k n", p=P)[
                            :, :, i * P:(i + 1) * P])
                    ph2 = pdp.tile([P, W2C], fp32, tag="ph2")
                    for ks in range(8):
                        nc.tensor.matmul(ph2[:], lhsT=ht[:, ks, :],
                                         rhs=w2_sb[:, ks, :],
                                         start=(ks == 0), stop=(ks == 7))
                    sh2 = pd.tile([P, DT2], fp32, tag="sh2")
                    nc.vector.tensor_copy(sh2[:, 0:OUT_DIM + 1], ph2[:, 0:OUT_DIM + 1])
                    nc.vector.memset(sh2[:, OUT_DIM + 1:DT2], 0.0)
                    nc.vector.tensor_copy(ad2_sb[:, i:i + 1],
                                          ph2[:, OUT_DIM + 1:OUT_DIM + 2])
                    nc.sync.dma_start(shard2[i * P:(i + 1) * P, :], sh2[:])

            nc.gpsimd.collective_compute(
                "AllGather", Alu.bypass,
                replica_groups=[list(range(NCORES))],
                ins=[shard2.opt()], outs=[table2.opt()],
            )

            # ================= phase F: gat2 aggregation ==================
            with (
                tc.tile_pool(name="pf_g", bufs=4) as pg2,
                tc.tile_pool(name="pf_m", bufs=4) as pm2,
                tc.tile_pool(name="pf_s", bufs=4) as psd2,
                tc.tile_pool(name="pf_q", bufs=4) as pq2,
                tc.tile_pool(name="pf_w", bufs=2) as pw2,
                tc.tile_pool(name="pf_po", bufs=2, space="PSUM") as ppo2,
                tc.tile_pool(name="pf_sc", bufs=2, space="PSUM") as psc2,
            ):
                for w in range(NW):
                    pso = ppo2.tile([P, OUT_DIM + 4], fp32, tag="pso")
                    for t in range(F):
                        tg = w * F + t
                        g = pg2.tile([P, DT2], fp32, tag="g2")
                        nc.gpsimd.indirect_dma_start(
                            out=g[:], out_offset=None, in_=table2[:],
                            in_offset=bass.IndirectOffsetOnAxis(
                                ap=sidx_sb[:, tg:tg + 1], axis=0),
                        )
                        sb2 = psd2.tile([P, 2, P], bf16, tag="sboth2")
                        nc.sync.dma_start(
                            sb2[:], s_both.ap()[tg].rearrange("j p c -> p j c"))
                        sed = sb2[:, 0, :]
                        sde = sb2[:, 1, :]

                        pead = psc2.tile([P, 4], fp32, tag="sc2")
                        nc.tensor.matmul(pead[:, 0:1], lhsT=sde[:],
                                         rhs=ad2_sb[:, w:w + 1], start=True, stop=True)
                        q = pq2.tile([P, 1], fp32, tag="qa")
                        q2 = pq2.tile([P, 1], fp32, tag="qb")
                        nc.vector.tensor_add(q[:], g[:, OUT_DIM:OUT_DIM + 1],
                                             pead[:, 0:1])
                        nc.vector.tensor_scalar_mul(q2[:], q[:], NEG_SLOPE)
                        nc.vector.tensor_tensor(q[:], q[:], q2[:], op=Alu.max)
                        msg = pm2.tile([P, OUT_DIM + 1], bf16, tag="msg2")
                        nc.scalar.activation(msg[:, OUT_DIM:OUT_DIM + 1], q[:],
                                             Act.Exp)
                        nc.vector.tensor_tensor(
                            out=msg[:, 0:OUT_DIM],
                            in0=g[:, 0:OUT_DIM],
                            in1=msg[:, OUT_DIM:OUT_DIM + 1].to_broadcast(
                                [P, OUT_DIM]),
                            op=Alu.mult,
                        )
                        nc.tensor.matmul(pso[:, 0:OUT_DIM + 1], lhsT=sed[:],
                                         rhs=msg[:], start=(t == 0), stop=(t == F - 1))
                    rd2 = pq2.tile([P, 1], fp32, tag="rd2")
                    nc.vector.tensor_scalar_add(rd2[:], pso[:, OUT_DIM:OUT_DIM + 1],
                                                1e-16)
                    nc.vector.reciprocal(rd2[:], rd2[:])
                    ot = pw2.tile([P, OUT_DIM], fp32, tag="ot")
                    nc.vector.tensor_tensor(
                        out=ot[:], in0=pso[:, 0:OUT_DIM],
                        in1=rd2[:].to_broadcast([P, OUT_DIM]), op=Alu.mult)
                    nc.vector.tensor_add(ot[:], ot[:], b2_sb[:])
                    nc.sync.dma_start(out.ap()[w * P:(w + 1) * P, :], ot[:])

    nc.compile()
    return nc


_CACHE = {}


def kernel(**inputs) -> np.ndarray:
    from concourse.bass_utils import run_bass_kernel_spmd

    pre = preprocess(**inputs)
    F = pre["F"]
    if F not in _CACHE:
        _CACHE[F] = build_program(F)
    nc = _CACHE[F]
    res = run_bass_kernel_spmd(nc, pre["in_maps"], core_ids=list(range(NCORES)))
    full = np.concatenate([r["out"] for r in res.results], axis=0)  # [NT, 128]
    return np.ascontiguousarray(full[pre["padded"]]).astype(np.float32)



# revision 3
# speedup vs baseline: 1.2883x; 1.2883x over previous
"""Trainium2 Bass kernel for nn_ConversationGNN (2-layer GAT, 50K nodes / 500K edges).

Strategy (8 NeuronCores, SPMD, one program):
  - Host: relabel nodes so each core owns 49 windows x 128 nodes, with edges
    (incl. self-loops) bin-packed so every window holds <= F*128 edges. All
    per-core structure (gather indices, per-edge dst slots) is plain input
    data -> a single uniform program runs on all 8 cores.
  - Device, per core:
      phase A (bf16): one fused matmul h1pre = [x|attr|1] @ (encW @ W1aug)
               per 128-node window (encoder and gat1 linear collapsed on the
               host; gat1 bias b1 folded into the table since softmax weights
               sum to 1). Windows are grouped into 4 chunks; each chunk's
               [rows x 1040] bf16 shard AllGathers into its chunk table as
               soon as the chunk's windows finish -> the collective overlaps
               the rest of phase A. The 4 chunk tables are allocated
               back-to-back (asserted), so one gather with global row ids
               spans all of them.
      phase C: per edge-tile (128 edges): indirect-DMA gather of source rows,
               one-hot scatter/broadcast matrices generated ON CHIP from the
               per-edge dst slot (iota + is_equal, PE transpose), attention
               softmax numerator+denominator accumulated in PSUM via matmuls.
               The tiny per-edge alpha ops are batched per window ([P, F*4]).
               gat2's linear layer is fused into the window drain (PE
               transposes + 8 accumulating matmuls) -> shard2/AllGather2
               (bf16, also chunked) with no h1 round-trip through DRAM.
      phase F: same aggregation for layer 2 on 144-byte bf16 rows.
  - Softmax max-subtraction is skipped: e = leaky(as+ad) is O(+-10) here, so
    exp() is well within f32/bf16 range and softmax is shift-invariant.
  - Cross-chunk-table gathers are ordered against ALL AllGather chunks via a
    zero-add chain: 4 bytes of each chunk table are read, multiplied by 0
    (integer ops, garbage-safe) and added to the gather offsets.

Self-contained: hardcodes all shapes; only needs numpy + the concourse tree
at /opt/trn_rl_repo (container-provided).
"""

import heapq
import sys

import ml_dtypes

import numpy as np

for _p in ("/opt/trn_rl_repo",):
    if _p not in sys.path:
        sys.path.insert(0, _p)

# problem constants
N = 50000
IN_DIM = 384
HID = 256
HEADS = 4
OUT_DIM = 128
NEG_SLOPE = 0.2

NCORES = 8
P = 128
NW = 49               # dst windows per core
NSH = NW * P          # 6272 padded nodes per core
NT = NCORES * NSH     # 50176 padded nodes total
D1 = HEADS * HID      # 1024
DT1 = 1040            # table1 bf16 row: [h1pre+b1 (1024) | a_src 4xf32 (8) | pad]
DT2 = 144             # table2 bf16 row: [h2pre+b2 (128) | a_src2 f32 (2) | pad]
W2C = OUT_DIM + 2     # 130: [W2 | W2@a_src2 | W2@a_dst2]

# AllGather chunking: windows grouped into 4 chunks; table rows are
# chunk-major (chunk, core, window-in-chunk, slot) so each chunk's AllGather
# is an independent collective that starts as soon as phase A (resp. C)
# finishes that chunk's windows.
CH_W = [13, 12, 12, 12]
CH_W0 = [0, 13, 25, 37]                      # first window of each chunk
CH_BASE = [0, 13 * P * NCORES, 25 * P * NCORES, 37 * P * NCORES]
CHUNK_OF_W = [0] * 13 + [1] * 12 + [2] * 12 + [3] * 12
PAD_SLOT = 300.0      # dst sentinel for padded edge slots -> all-zero one-hot


def _table_id(padded):
    """Map dst-padded id (core*NSH + w*128 + s) -> chunk-major table row."""
    c = padded // NSH
    r = padded % NSH
    w = r // P
    s = r % P
    k = np.asarray(CHUNK_OF_W)[w]
    base = np.asarray(CH_BASE)[k]
    w0 = np.asarray(CH_W0)[k]
    szk = np.asarray(CH_W)[k]
    return base + c * szk * P + (w - w0) * P + s


# ---------------------------------------------------------------- host side

def _pack_nodes(deg):
    """Assign nodes to (window, slot) so window edge-counts are balanced.

    Returns padded ids [N] (window*128 + slot) and per-window edge counts.
    """
    nwg = NCORES * NW
    order = np.argsort(-deg, kind="stable")
    heap = [(0, w) for w in range(nwg)]
    heapq.heapify(heap)
    slots_used = np.zeros(nwg, np.int64)
    edges_w = np.zeros(nwg, np.int64)
    assign_w = np.empty(N, np.int64)
    assign_s = np.empty(N, np.int64)
    for i in order:
        while True:
            _, w = heapq.heappop(heap)
            if slots_used[w] < P:
                break
        assign_w[i] = w
        assign_s[i] = slots_used[w]
        slots_used[w] += 1
        edges_w[w] += deg[i]
        heapq.heappush(heap, (int(edges_w[w]), w))
    return assign_w * P + assign_s, edges_w


def preprocess(x, node_attr, edge_index, enc_W, enc_b,
               W1, a_src1, a_dst1, b1, W2, a_src2, a_dst2, b2):
    bf16 = ml_dtypes.bfloat16
    x = np.asarray(x, np.float32)
    node_attr = np.asarray(node_attr, np.float32)
    ei = np.asarray(edge_index)
    src_all = np.concatenate([ei[0], np.arange(N, dtype=ei.dtype)]).astype(np.int64)
    dst_all = np.concatenate([ei[1], np.arange(N, dtype=ei.dtype)]).astype(np.int64)
    ne = src_all.shape[0]

    deg = np.bincount(dst_all, minlength=N)
    padded, edges_w = _pack_nodes(deg)
    F = int(np.ceil(edges_w.max() / P))
    T = NW * F

    tid = _table_id(padded)          # node -> chunk-major table row

    spad = padded[src_all]
    dpad = padded[dst_all]
    wg = dpad // P
    dst_rel = (dpad % P).astype(np.int64)

    order_e = np.argsort(wg, kind="stable")
    wg_s = wg[order_e]
    counts = np.bincount(wg_s, minlength=NCORES * NW)
    starts = np.zeros(NCORES * NW + 1, np.int64)
    starts[1:] = np.cumsum(counts)
    slot = np.arange(ne) - starts[wg_s]

    core_of = wg_s // NW
    w_loc = wg_s % NW
    tile_g = w_loc * F + slot // P
    e_in = slot % P

    SRC = np.zeros((NCORES, T, P), np.int32)      # table row of src per slot
    DREL = np.full((NCORES, T, P), PAD_SLOT, np.float32)
    SRC[core_of, tile_g, e_in] = tid[src_all[order_e]].astype(np.int32)
    DREL[core_of, tile_g, e_in] = dst_rel[order_e].astype(np.float32)

    # node features, padded-id order, with [x | attr | 1 | 0] layout (512)
    x2 = np.zeros((NT, 512), np.float32)
    x2[padded, 0:IN_DIM] = x
    x2[padded, IN_DIM:IN_DIM + 2] = node_attr
    x2[padded, IN_DIM + 2] = 1.0

    # fused encoder + gat1 weights: [x|attr|1] @ (encWaug @ W1aug)
    encWaug = np.zeros((512, IN_DIM), np.float32)
    encWaug[0:IN_DIM + 2] = np.asarray(enc_W, np.float32)
    encWaug[IN_DIM + 2] = np.asarray(enc_b, np.float32)

    W1 = np.asarray(W1, np.float32)
    asrc1t = np.einsum("fhc,hc->fh", W1.reshape(IN_DIM, HEADS, HID),
                       np.asarray(a_src1, np.float32))
    adst1t = np.einsum("fhc,hc->fh", W1.reshape(IN_DIM, HEADS, HID),
                       np.asarray(a_dst1, np.float32))
    W1aug = np.concatenate([W1, asrc1t, adst1t], axis=1)  # [384, 1032]
    wcomb = (encWaug.astype(np.float64) @ W1aug.astype(np.float64)
             ).astype(np.float32)                          # [512, 1032]
    wcomb[IN_DIM + 2, 0:D1] += np.asarray(b1, np.float32)  # fold b1 (sum a=1)

    W2 = np.asarray(W2, np.float32)
    w2aug = np.concatenate(
        [W2,
         W2 @ np.asarray(a_src2, np.float32)[0][:, None],
         W2 @ np.asarray(a_dst2, np.float32)[0][:, None]], axis=1)  # [1024, 130]

    b2rep = np.tile(np.asarray(b2, np.float32)[None, :], (P, 1))

    in_maps = []
    for c in range(NCORES):
        in_maps.append({
            "x2T": np.ascontiguousarray(
                x2[c * NSH:(c + 1) * NSH].T).astype(bf16),
            "wcomb": wcomb.astype(bf16),
            "w2aug": w2aug.astype(bf16),
            "b2rep": b2rep,
            "srcidx": np.ascontiguousarray(SRC[c].T),       # [128, T] i32
            "dstrel": np.ascontiguousarray(DREL[c].T),      # [128, T] f32
        })
    return {"in_maps": in_maps, "F": F, "padded": padded}


# -------------------------------------------------------------- bass program

def build_program(F):
    import concourse.bacc as bacc
    import concourse.bass as bass
    import concourse.mybir as mybir
    import concourse.tile as tile
    from concourse.masks import make_identity

    fp32 = mybir.dt.float32
    bf16 = mybir.dt.bfloat16
    i32 = mybir.dt.int32
    Alu = mybir.AluOpType
    Act = mybir.ActivationFunctionType
    T = NW * F

    nc = bacc.Bacc("TRN2", target_bir_lowering=False, debug=False,
                   enable_asserts=False, num_devices=NCORES)

    x2T = nc.dram_tensor("x2T", [512, NSH], bf16, kind="ExternalInput")
    wcomb = nc.dram_tensor("wcomb", [512, 1032], bf16, kind="ExternalInput")
    w2aug = nc.dram_tensor("w2aug", [D1, W2C], bf16, kind="ExternalInput")
    b2rep = nc.dram_tensor("b2rep", [P, OUT_DIM], fp32, kind="ExternalInput")
    srcidx = nc.dram_tensor("srcidx", [P, T], i32, kind="ExternalInput")
    dstrel = nc.dram_tensor("dstrel", [P, T], fp32, kind="ExternalInput")
    out = nc.dram_tensor("out", [NSH, OUT_DIM], fp32, kind="ExternalOutput")

    # internal DRAM: per-chunk shards and back-to-back chunk tables
    shard1 = []
    shard2 = []
    for k, szk in enumerate(CH_W):
        shard1.append(nc.dram_tensor(f"shard1_{k}", [szk * P, DT1], bf16,
                                     kind="Internal"))
        shard2.append(nc.dram_tensor(f"shard2_{k}", [szk * P, DT2], bf16,
                                     kind="Internal"))
    table1 = []
    table2 = []
    for k, szk in enumerate(CH_W):
        table1.append(nc.dram_tensor(
            f"table1_{k}", [NCORES * szk * P, DT1], bf16,
            kind="Internal", addr_space="Shared"))
    for k, szk in enumerate(CH_W):
        table2.append(nc.dram_tensor(
            f"table2_{k}", [NCORES * szk * P, DT2], bf16,
            kind="Internal", addr_space="Shared"))
    # the cross-chunk gather assumes the 4 chunk tables sit back-to-back
    for tabs, dt in ((table1, DT1), (table2, DT2)):
        addrs = [nc.lookup_mls(t).memorylocations[0].addr for t in tabs]
        for k in range(3):
            assert addrs[k + 1] == addrs[k] + NCORES * CH_W[k] * P * dt * 2, \
                ("chunk tables not contiguous", addrs, dt)

    rg = [list(range(NCORES))]

    with tile.TileContext(nc) as tc:
        with tc.tile_pool(name="const", bufs=1) as constp:
            # ---- persistent SBUF data
            wc_sb = constp.tile([P, 4, 1032], bf16)
            nc.sync.dma_start(wc_sb[:], wcomb.ap().rearrange("(k p) f -> p k f", p=P))
            w2_sb = constp.tile([P, 8, W2C], bf16)
            nc.sync.dma_start(w2_sb[:], w2aug.ap().rearrange("(k p) f -> p k f", p=P))
            b2_sb = constp.tile([P, OUT_DIM], fp32)
            nc.sync.dma_start(b2_sb[:], b2rep.ap())
            sidx_raw = constp.tile([P, T], i32)
            nc.sync.dma_start(sidx_raw[:], srcidx.ap())
            drel_sb = constp.tile([P, T], fp32)
            nc.sync.dma_start(drel_sb[:], dstrel.ap())
            iota_i = constp.tile([P, P], i32)
            nc.gpsimd.iota(iota_i[:], pattern=[[1, P]], base=0,
                           channel_multiplier=0)
            iota_f = constp.tile([P, P], fp32)
            nc.vector.tensor_copy(iota_f[:], iota_i[:])
            identb = constp.tile([P, P], bf16)
            make_identity(nc, identb[:])
            ad1_sb = constp.tile([P, NW, 4], bf16)
            ad2_sb = constp.tile([P, NW], fp32)
            sidx1 = constp.tile([P, T], i32)
            sidx2 = constp.tile([P, T], i32)

            # ================= phase A: fused enc+gat1 linear =============
            with (
                tc.tile_pool(name="pa_sb", bufs=3) as pa,
                tc.tile_pool(name="pa_ps", bufs=2, space="PSUM") as pap,
            ):
                for i in range(NW):
                    k = CHUNK_OF_W[i]
                    xt = pa.tile([P, 4, P], bf16, tag="xt")
                    nc.sync.dma_start(
                        xt[:],
                        x2T.ap().rearrange("(k p) n -> p k n", p=P)[
                            :, :, i * P:(i + 1) * P],
                    )
                    ph1a = pap.tile([P, 512], fp32, tag="ph1a")
                    ph1b = pap.tile([P, 512], fp32, tag="ph1b")
                    ph1c = pap.tile([P, 8], fp32, tag="ph1c")
                    for ks in range(4):
                        st, sp = (ks == 0), (ks == 3)
                        nc.tensor.matmul(ph1a[:], lhsT=xt[:, ks, :],
                                         rhs=wc_sb[:, ks, 0:512],
                                         start=st, stop=sp)
                        nc.tensor.matmul(ph1b[:], lhsT=xt[:, ks, :],
                                         rhs=wc_sb[:, ks, 512:1024],
                                         start=st, stop=sp)
                        nc.tensor.matmul(ph1c[:], lhsT=xt[:, ks, :],
                                         rhs=wc_sb[:, ks, 1024:1032],
                                         start=st, stop=sp)
                    sh1 = pa.tile([P, DT1], bf16, tag="sh1")
                    nc.vector.tensor_copy(sh1[:, 0:512], ph1a[:])
                    nc.vector.tensor_copy(sh1[:, 512:1024], ph1b[:])
                    nc.vector.tensor_copy(
                        sh1[:, 1024:1032].bitcast(fp32), ph1c[:, 0:4])
                    nc.vector.memset(sh1[:, 1032:1040], 0.0)
                    nc.vector.tensor_copy(ad1_sb[:, i, :], ph1c[:, 4:8])
                    r0 = (i - CH_W0[k]) * P
                    nc.sync.dma_start(shard1[k].ap()[r0:r0 + P, :], sh1[:])
                    if i == CH_W0[k] + CH_W[k] - 1:
                        nc.gpsimd.collective_compute(
                            "AllGather", Alu.bypass, replica_groups=rg,
                            ins=[shard1[k].ap()], outs=[table1[k].ap()],
                        )

            # gather offsets usable only after ALL AG1 chunks: zero-add chain
            z1 = constp.tile([P, 3], i32)
            for k in range(3):
                nc.sync.dma_start(z1[:, k:k + 1],
                                  table1[k + 1].ap()[0:P, 0:2].bitcast(i32))
            zm1 = constp.tile([P, 3], i32)
            nc.vector.tensor_scalar_mul(zm1[:], z1[:], 0)
            nc.vector.tensor_tensor(out=sidx1[:], in0=sidx_raw[:],
                                    in1=zm1[:, 0:1].to_broadcast([P, T]),
                                    op=Alu.add)

            # ================= phase C: gat1 aggregation + gat2 linear ====
            with (
                tc.tile_pool(name="pc_g", bufs=2) as pg,
                tc.tile_pool(name="pc_sed", bufs=2) as psed,
                tc.tile_pool(name="pc_sde", bufs=3) as psde,
                tc.tile_pool(name="pc_m", bufs=3) as pm,
                tc.tile_pool(name="pc_q", bufs=2) as pq,
                tc.tile_pool(name="pc_w", bufs=2) as pw,
                tc.tile_pool(name="pc_p0", bufs=2, space="PSUM") as pp0,
                tc.tile_pool(name="pc_p1", bufs=2, space="PSUM") as pp1,
                tc.tile_pool(name="pc_pc", bufs=2, space="PSUM") as ppc,
                tc.tile_pool(name="pc_tr", bufs=2, space="PSUM") as ptr,
            ):
                for w in range(NW):
                    k = CHUNK_OF_W[w]
                    po0 = pp0.tile([P, 512], fp32, tag="po0")
                    po1 = pp1.tile([P, 512], fp32, tag="po1")
                    poc = ppc.tile([P, 512], fp32, tag="poc")
                    sedw = psed.tile([P, F, P], bf16, tag="sedw")
                    asw = pq.tile([P, F, 8], bf16, tag="asw")
                    gts = []
                    for t in range(F):
                        tg = w * F + t
                        g = pg.tile([P, DT1], bf16, tag=f"g{t}")
                        nc.gpsimd.indirect_dma_start(
                            out=g[:], out_offset=None, in_=table1[0].ap(),
                            in_offset=bass.IndirectOffsetOnAxis(
                                ap=sidx1[:, tg:tg + 1], axis=0),
                        )
                        gts.append(g)
                        nc.vector.tensor_tensor(
                            out=sedw[:, t, :],
                            in0=drel_sb[:, tg:tg + 1].to_broadcast([P, P]),
                            in1=iota_f[:], op=Alu.is_equal)
                        ptile = ptr.tile([P, P], bf16, tag="ptile")
                        nc.tensor.transpose(ptile[:], sedw[:, t, :], identb[:])
                        sde = psde.tile([P, P], bf16, tag="sde")
                        nc.scalar.activation(sde[:], ptile[:], Act.Copy)
                        nc.tensor.matmul(
                            poc[:, 8 + 4 * t:12 + 4 * t], lhsT=sde[:],
                            rhs=ad1_sb[:, w, :], start=True, stop=True,
                            skip_group_check=True)
                        nc.scalar.activation(asw[:, t, :], g[:, 1024:1032],
                                             Act.Copy)
                    # batched per-window alpha math
                    q = pq.tile([P, 4 * F], fp32, tag="q")
                    nc.vector.tensor_tensor(
                        out=q[:],
                        in0=asw[:].bitcast(fp32).rearrange("p f c -> p (f c)"),
                        in1=poc[:, 8:8 + 4 * F], op=Alu.add)
                    q2 = pq.tile([P, 4 * F], fp32, tag="q2")
                    nc.vector.tensor_scalar_mul(q2[:], q[:], NEG_SLOPE)
                    nc.vector.tensor_tensor(q[:], q[:], q2[:], op=Alu.max)
                    eq = pq.tile([P, 4 * F], bf16, tag="eq")
                    nc.scalar.activation(eq[:], q[:], Act.Exp)
                    for t in range(F):
                        g = gts[t]
                        msg = pm.tile([P, D1], bf16, tag="msg")
                        nc.vector.tensor_tensor(
                            out=msg[:].rearrange("p (h c) -> p h c", h=HEADS),
                            in0=g[:, 0:D1].rearrange("p (h c) -> p h c", h=HEADS),
                            in1=eq[:, 4 * t:4 * t + 4][:, :, None].to_broadcast(
                                [P, HEADS, HID]),
                            op=Alu.mult)
                        st, sp = (t == 0), (t == F - 1)
                        nc.tensor.matmul(po0[:], lhsT=sedw[:, t, :],
                                         rhs=msg[:, 0:512], start=st, stop=sp)
                        nc.tensor.matmul(po1[:], lhsT=sedw[:, t, :],
                                         rhs=msg[:, 512:1024], start=st, stop=sp)
                        nc.tensor.matmul(poc[:, 0:4], lhsT=sedw[:, t, :],
                                         rhs=eq[:, 4 * t:4 * t + 4],
                                         start=st, stop=sp,
                                         skip_group_check=True)
                    # ---- window drain: normalize, ELU, gat2 linear
                    rden = pq.tile([P, 4], fp32, tag="rden")
                    nc.vector.tensor_scalar_add(rden[:], poc[:, 0:4], 1e-16)
                    nc.vector.reciprocal(rden[:], rden[:])
                    z = pw.tile([P, D1], fp32, tag="z")
                    nc.vector.tensor_tensor(
                        out=z[:, 0:512].rearrange("p (h c) -> p h c", h=2),
                        in0=po0[:].rearrange("p (h c) -> p h c", h=2),
                        in1=rden[:, 0:2][:, :, None].to_broadcast([P, 2, HID]),
                        op=Alu.mult)
                    nc.vector.tensor_tensor(
                        out=z[:, 512:1024].rearrange("p (h c) -> p h c", h=2),
                        in0=po1[:].rearrange("p (h c) -> p h c", h=2),
                        in1=rden[:, 2:4][:, :, None].to_broadcast([P, 2, HID]),
                        op=Alu.mult)
                    # ELU(z) = (max(z,0) - 1) + exp(min(z,0))
                    zm = pw.tile([P, D1], fp32, tag="zmt")
                    nc.vector.tensor_scalar_min(zm[:], z[:], 0.0)
                    em = pw.tile([P, D1], fp32, tag="em")
                    nc.scalar.activation(em[:], zm[:], Act.Exp)
                    nc.vector.tensor_scalar(out=z[:], in0=z[:], scalar1=0.0,
                                            scalar2=-1.0, op0=Alu.max,
                                            op1=Alu.add)
                    h1 = pw.tile([P, D1], bf16, tag="h1")
                    nc.vector.tensor_tensor(h1[:], z[:], em[:], op=Alu.add)
                    # gat2 linear fused: ph2 = h1 @ w2aug via PE transposes
                    for fb in range(8):
                        ptile = ptr.tile([P, P], bf16, tag="ptile")
                        nc.tensor.transpose(ptile[:], h1[:, fb * P:(fb + 1) * P],
                                            identb[:])
                        trs = psde.tile([P, P], bf16, tag="sde")
                        nc.scalar.activation(trs[:], ptile[:], Act.Copy)
                        nc.tensor.matmul(poc[:, 128:128 + W2C], lhsT=trs[:],
                                         rhs=w2_sb[:, fb, :],
                                         start=(fb == 0), stop=(fb == 7),
                                         skip_group_check=True)
                    sh2 = pw.tile([P, DT2], bf16, tag="sh2")
                    nc.vector.tensor_add(sh2[:, 0:OUT_DIM], poc[:, 128:256],
                                         b2_sb[:])
                    nc.vector.tensor_copy(
                        sh2[:, OUT_DIM:OUT_DIM + 2].bitcast(fp32),
                        poc[:, 256:257])
                    nc.vector.memset(sh2[:, OUT_DIM + 2:DT2], 0.0)
                    nc.vector.tensor_copy(ad2_sb[:, w:w + 1], poc[:, 257:258])
                    r0 = (w - CH_W0[k]) * P
                    nc.sync.dma_start(shard2[k].ap()[r0:r0 + P, :], sh2[:])
                    if w == CH_W0[k] + CH_W[k] - 1:
                        nc.gpsimd.collective_compute(
                            "AllGather", Alu.bypass, replica_groups=rg,
                            ins=[shard2[k].ap()], outs=[table2[k].ap()],
                        )

            z2 = constp.tile([P, 3], i32)
            for k in range(3):
                nc.sync.dma_start(z2[:, k:k + 1],
                                  table2[k + 1].ap()[0:P, 0:2].bitcast(i32))
            zm2 = constp.tile([P, 3], i32)
            nc.vector.tensor_scalar_mul(zm2[:], z2[:], 0)
            nc.vector.tensor_tensor(out=sidx2[:], in0=sidx_raw[:],
                                    in1=zm2[:, 0:1].to_broadcast([P, T]),
                                    op=Alu.add)

            # ================= phase F: gat2 aggregation ==================
            with (
                tc.tile_pool(name="pf_g", bufs=2) as pg2,
                tc.tile_pool(name="pf_sed", bufs=2) as psed2,
                tc.tile_pool(name="pf_sde", bufs=3) as psde2,
                tc.tile_pool(name="pf_m", bufs=3) as pm2,
                tc.tile_pool(name="pf_q", bufs=2) as pq2,
                tc.tile_pool(name="pf_w", bufs=2) as pw2,
                tc.tile_pool(name="pf_ps", bufs=2, space="PSUM") as ppf,
                tc.tile_pool(name="pf_tr", bufs=2, space="PSUM") as ptr2,
            ):
                for w in range(NW):
                    pof = ppf.tile([P, 512], fp32, tag="pof")
                    sedw = psed2.tile([P, F, P], bf16, tag="sedw2")
                    asw = pq2.tile([P, F, 2], bf16, tag="asw2")
                    ad2b = pq2.tile([P, 1], bf16, tag="ad2b")
                    nc.vector.tensor_copy(ad2b[:], ad2_sb[:, w:w + 1])
                    gts = []
                    for t in range(F):
                        tg = w * F + t
                        g = pg2.tile([P, DT2], bf16, tag=f"g2_{t}")
                        nc.gpsimd.indirect_dma_start(
                            out=g[:], out_offset=None, in_=table2[0].ap(),
                            in_offset=bass.IndirectOffsetOnAxis(
                                ap=sidx2[:, tg:tg + 1], axis=0),
                        )
                        gts.append(g)
                        nc.vector.tensor_tensor(
                            out=sedw[:, t, :],
                            in0=drel_sb[:, tg:tg + 1].to_broadcast([P, P]),
                            in1=iota_f[:], op=Alu.is_equal)
                        ptile = ptr2.tile([P, P], bf16, tag="ptile2")
                        nc.tensor.transpose(ptile[:], sedw[:, t, :], identb[:])
                        sde = psde2.tile([P, P], bf16, tag="sde2")
                        nc.scalar.activation(sde[:], ptile[:], Act.Copy)
                        nc.tensor.matmul(
                            pof[:, 256 + t:257 + t], lhsT=sde[:],
                            rhs=ad2b[:], start=True, stop=True,
                            skip_group_check=True)
                        nc.scalar.activation(asw[:, t, :], g[:, 128:130],
                                             Act.Copy)
                    q = pq2.tile([P, F], fp32, tag="qf")
                    nc.vector.tensor_tensor(
                        out=q[:],
                        in0=asw[:].bitcast(fp32).rearrange("p f c -> p (f c)"),
                        in1=pof[:, 256:256 + F], op=Alu.add)
                    q2 = pq2.tile([P, F], fp32, tag="qf2")
                    nc.vector.tensor_scalar_mul(q2[:], q[:], NEG_SLOPE)
                    nc.vector.tensor_tensor(q[:], q[:], q2[:], op=Alu.max)
                    eq = pq2.tile([P, F], bf16, tag="eqf")
                    nc.scalar.activation(eq[:], q[:], Act.Exp)
                    for t in range(F):
                        g = gts[t]
                        msg = pm2.tile([P, OUT_DIM + 1], bf16, tag="msg2")
                        nc.vector.tensor_tensor(
                            out=msg[:, 0:OUT_DIM], in0=g[:, 0:OUT_DIM],
                            in1=eq[:, t:t + 1].to_broadcast([P, OUT_DIM]),
                            op=Alu.mult)
                        nc.scalar.activation(msg[:, OUT_DIM:OUT_DIM + 1],
                                             eq[:, t:t + 1], Act.Copy)
                        nc.tensor.matmul(pof[:, 0:OUT_DIM + 1],
                                         lhsT=sedw[:, t, :], rhs=msg[:],
                                         start=(t == 0), stop=(t == F - 1),
                                         skip_group_check=True)
                    rd2 = pq2.tile([P, 1], fp32, tag="rd2")
                    nc.vector.tensor_scalar_add(rd2[:],
                                                pof[:, OUT_DIM:OUT_DIM + 1],
                                                1e-16)
                    nc.vector.reciprocal(rd2[:], rd2[:])
                    ot = pw2.tile([P, OUT_DIM], fp32, tag="ot")
                    nc.vector.tensor_tensor(
                        out=ot[:], in0=pof[:, 0:OUT_DIM],
                        in1=rd2[:].to_broadcast([P, OUT_DIM]), op=Alu.mult)
                    nc.sync.dma_start(out.ap()[w * P:(w + 1) * P, :], ot[:])

    nc.compile()
    return nc


_CACHE = {}


def kernel(**inputs) -> np.ndarray:
    from concourse.bass_utils import run_bass_kernel_spmd

    pre = preprocess(**inputs)
    F = pre["F"]
    if F not in _CACHE:
        _CACHE[F] = build_program(F)
    nc = _CACHE[F]
    res = run_bass_kernel_spmd(nc, pre["in_maps"], core_ids=list(range(NCORES)))
    full = np.concatenate([r["out"] for r in res.results], axis=0)  # [NT, 128]
    return np.ascontiguousarray(full[pre["padded"]]).astype(np.float32)
